# revision 6
# baseline (speedup 1.0000x reference)
"""Trainium2 Bass kernel for nn_ChunkwiseRecurrentAttentionCell.

Math (per (b,h) slice; T=256, Dk=Dv=128):
    gc = cumsum(g);  A = tril(beta_i exp(gc_i-gc_j) k_i.k_j, -1)
    v_new = (I+A)^{-1} (beta v - beta exp(gc) (k @ S0))
    out   = exp(gc) (q@S0) + (tril(exp(gc_i-gc_j),0) * (q k^T)) @ v_new
    S_new = exp(gc_T) S0 + k^T (v_new * exp(gc_T - gc))

Implemented as a chunked recurrence (2 chunks of 128) so all per-chunk exp
ratios are bounded by e^6.4 (fp16-safe).  The triangular solve uses an
8-term Neumann product form  (I+X^4)(I+X^2)(I+X), X = -A_chunk, with dual
power chains (both X^p and its transpose built by matmuls from masked
scalings of the symmetric K K^T — no big transposes needed).  All matmul
operands are fp16 (PE runs fp16 at 1 cycle/row vs fp32's 4); accumulation
is fp32 in PSUM.  Relative error vs the fp32 reference ~ 4e-4.

Sharding: (B,H) flattened to 512 independent slices, 64 per core across
8 NeuronCores (data parallel, no collectives).
"""

import os
import numpy as np

import concourse.bass as bass
import concourse.mybir as mybir
from concourse import bacc
from concourse.tile import TileContext
from concourse.masks import (
    make_identity,
    make_lower_triangular,
    make_upper_triangular,
)

B, H, T, DK, DV = 16, 32, 256, 128, 128
N_CORES = 8
N_SLICES = (B * H) // N_CORES  # 64 per core
CH = 128  # chunk length
N_CHUNKS = T // CH
LEVELS = 3  # Neumann product-form levels -> 2^3 = 8 series terms

F32 = mybir.dt.float32
MM_DT = mybir.dt.float16

_ALU = mybir.AluOpType
_ACTF = mybir.ActivationFunctionType


def build_nc(n_slices: int = N_SLICES):
    nc = bacc.Bacc("TRN2", target_bir_lowering=False)

    dq = nc.dram_tensor("q", [n_slices, T, DK], F32, kind="ExternalInput")
    dk = nc.dram_tensor("k", [n_slices, T, DK], F32, kind="ExternalInput")
    dv = nc.dram_tensor("v", [n_slices, T, DV], F32, kind="ExternalInput")
    dg = nc.dram_tensor("g", [n_slices, T], F32, kind="ExternalInput")
    db = nc.dram_tensor("beta", [n_slices, T], F32, kind="ExternalInput")
    ds0 = nc.dram_tensor("s0", [n_slices, DK, DV], F32, kind="ExternalInput")
    dout = nc.dram_tensor("out", [n_slices, T, DV], F32, kind="ExternalOutput")
    dsn = nc.dram_tensor("s_new", [n_slices, DK, DV], F32, kind="ExternalOutput")

    with TileContext(nc) as tc:
        with (
            tc.tile_pool(name="const", bufs=1) as cpool,
            tc.tile_pool(name="io", bufs=3) as iop,
            tc.tile_pool(name="ops", bufs=3) as opp,
            tc.tile_pool(name="state", bufs=2) as stp,
            tc.tile_pool(name="ps", bufs=1, space="PSUM") as psp,
        ):
            # ---------------- constants ----------------
            ident16 = cpool.tile([128, 128], MM_DT)
            make_identity(nc, ident16)
            ident32 = cpool.tile([128, 128], F32)
            make_identity(nc, ident32)
            mask_sl = cpool.tile([128, 128], F32)  # strict lower ones
            make_lower_triangular(nc, mask_sl, val=1.0, diag=False)
            mask_su = cpool.tile([128, 128], F32)  # strict upper ones
            make_upper_triangular(nc, mask_su, val=1.0, diag=False)
            mask_ui = cpool.tile([128, 128], F32)  # upper ones incl diag
            make_upper_triangular(nc, mask_ui, val=1.0, diag=True)

            # ---------------- per-core setup: gate vectors ----------------
            gt = cpool.tile([n_slices, T], F32)
            nc.sync.dma_start(gt[:], dg[:])
            bt = cpool.tile([n_slices, T], F32)
            nc.sync.dma_start(bt[:], db[:])
            gct = cpool.tile([n_slices, T], F32)
            nc.vector.tensor_tensor_scan(
                gct[:], gt[:], gt[:], 0.0, op0=_ALU.add, op1=_ALU.bypass
            )
            gcl1 = cpool.tile([n_slices, CH], F32)
            nc.vector.tensor_scalar(
                gcl1[:], gct[:, CH : 2 * CH], gct[:, CH - 1 : CH], None,
                op0=_ALU.subtract,
            )

            # per chunk: r, 1/r, -beta*r  in [n_slices, CH]; then transpose to
            # [CH, n_slices] so columns are per-slice partition-scalars.
            rT, irT, nbrT, bT, ET = [], [], [], [], []
            for c in range(N_CHUNKS):
                gcl = gct[:, 0:CH] if c == 0 else gcl1[:]
                r_c = cpool.tile([n_slices, CH], F32, name=f"r_{c}")
                nc.scalar.activation(r_c[:], gcl, _ACTF.Exp)
                ir_c = cpool.tile([n_slices, CH], F32, name=f"ir_{c}")
                nc.scalar.activation(ir_c[:], gcl, _ACTF.Exp, scale=-1.0)
                nbr_c = cpool.tile([n_slices, CH], F32, name=f"nbr_{c}")
                nc.vector.scalar_tensor_tensor(
                    nbr_c[:],
                    bt[:, c * CH : (c + 1) * CH],
                    -1.0,
                    r_c[:],
                    op0=_ALU.mult,
                    op1=_ALU.mult,
                )
                outs = []
                for src, nm in (
                    (r_c[:], "rT"),
                    (ir_c[:], "irT"),
                    (nbr_c[:], "nbrT"),
                    (bt[:, c * CH : (c + 1) * CH], "bT"),
                ):
                    pst = psp.tile([CH, n_slices], F32, name=f"pst_{nm}{c}", tag="ps_t", bufs=3)
                    nc.tensor.transpose(pst[:], src, ident32[0:n_slices, 0:n_slices])
                    dst = cpool.tile([CH, n_slices], F32, name=f"{nm}_{c}")
                    nc.scalar.copy(dst[:], pst[:])
                    outs.append(dst)
                rT.append(outs[0])
                irT.append(outs[1])
                nbrT.append(outs[2])
                bT.append(outs[3])
                ps_e = psp.tile([1, n_slices], F32, name=f"ps_e{c}", tag="ps_t", bufs=3)
                nc.tensor.transpose(
                    ps_e[:], r_c[:, CH - 1 : CH], ident32[0:n_slices, 0:n_slices]
                )
                e_row = cpool.tile([1, n_slices], F32, name=f"e_row_{c}")
                nc.scalar.copy(e_row[:], ps_e[:])
                e_c = cpool.tile([CH, n_slices], F32, name=f"ET_{c}")
                nc.gpsimd.partition_broadcast(e_c[:], e_row[0:1, :])
                ET.append(e_c)

            # ---------------- main loop over slices ----------------
            for s in range(n_slices):
                s_cur = None
                for c in range(N_CHUNKS):
                    tsl = slice(c * CH, (c + 1) * CH)
                    q_c = iop.tile([CH, DK], F32, name="q_c")
                    nc.sync.dma_start(q_c[:], dq[s, tsl, :])
                    k_c = iop.tile([CH, DK], F32, name="k_c")
                    nc.sync.dma_start(k_c[:], dk[s, tsl, :])
                    v_c = iop.tile([CH, DV], F32, name="v_c")
                    nc.sync.dma_start(v_c[:], dv[s, tsl, :])
                    if c == 0:
                        s_f32 = iop.tile([DK, DV], F32, name="s_f32")
                        nc.sync.dma_start(s_f32[:], ds0[s, :, :])
                        s_cur = stp.tile([DK, DV], MM_DT, name="s_cur")
                        nc.gpsimd.tensor_copy(s_cur[:], s_f32[:])

                    # scaled copies (fp16)
                    qr = opp.tile([CH, DK], MM_DT, name="qr")
                    nc.gpsimd.tensor_scalar_mul(qr[:], q_c[:], rT[c][:, s : s + 1])
                    knbr = opp.tile([CH, DK], MM_DT, name="knbr")
                    nc.gpsimd.tensor_scalar_mul(knbr[:], k_c[:], nbrT[c][:, s : s + 1])
                    kir = opp.tile([CH, DK], MM_DT, name="kir")
                    nc.gpsimd.tensor_scalar_mul(kir[:], k_c[:], irT[c][:, s : s + 1])

                    # transposes (PE) + copies (ACT)
                    qT = opp.tile([DK, CH], MM_DT, name="qT")
                    kTn = opp.tile([DK, CH], MM_DT, name="kTn")
                    kTi = opp.tile([DK, CH], MM_DT, name="kTi")
                    for src, dst, nm in ((qr, qT, "q"), (knbr, kTn, "n"), (kir, kTi, "i")):
                        ps_t = psp.tile([DK, CH], MM_DT, name=f"ps_t{nm}", tag="ps_t", bufs=3)
                        nc.tensor.transpose(ps_t[:], src[:], ident16[:])
                        nc.scalar.copy(dst[:], ps_t[:])

                    # Y = beta*v + (knbr @ S)     [= beta*v - beta*r*(k@S)]
                    ps_y = psp.tile([CH, DV], F32, name="ps_y", tag="mm", bufs=3)
                    nc.tensor.matmul(ps_y[:], kTn[:], s_cur[:])
                    z = opp.tile([CH, DV], MM_DT, name="z_it", tag="z", bufs=4)
                    nc.vector.scalar_tensor_tensor(
                        z[:], v_c[:], bT[c][:, s : s + 1], ps_y[:],
                        op0=_ALU.mult, op1=_ALU.add,
                    )

                    # B0 = -A = strict_tril(knbr @ kir^T); C0 = B0^T
                    ps_a = psp.tile([CH, CH], F32, name="ps_a", tag="mm", bufs=3)
                    nc.tensor.matmul(ps_a[:], kTn[:], kTi[:])
                    b0 = opp.tile([CH, CH], MM_DT, name="b0")
                    nc.vector.tensor_tensor(b0[:], ps_a[:], mask_sl[:], _ALU.mult)
                    ps_at = psp.tile([CH, CH], F32, name="ps_at", tag="mm", bufs=3)
                    nc.tensor.matmul(ps_at[:], kTi[:], kTn[:])
                    c0 = opp.tile([CH, CH], MM_DT, name="c0")
                    nc.vector.tensor_tensor(c0[:], ps_at[:], mask_su[:], _ALU.mult)

                    # dual chain: B1 = B0@B0, C1 = C0@C0, C2 = C1@C1
                    ps_b1 = psp.tile([CH, CH], F32, name="ps_b1", tag="mm", bufs=3)
                    nc.tensor.matmul(ps_b1[:], c0[:], b0[:])
                    b1 = opp.tile([CH, CH], MM_DT, name="b1")
                    nc.scalar.copy(b1[:], ps_b1[:])
                    ps_c1 = psp.tile([CH, CH], F32, name="ps_c1", tag="mm", bufs=3)
                    nc.tensor.matmul(ps_c1[:], b0[:], c0[:])
                    c1 = opp.tile([CH, CH], MM_DT, name="c1")
                    nc.scalar.copy(c1[:], ps_c1[:])
                    ps_c2 = psp.tile([CH, CH], F32, name="ps_c2", tag="mm", bufs=3)
                    nc.tensor.matmul(ps_c2[:], b1[:], c1[:])
                    c2 = opp.tile([CH, CH], MM_DT, name="c2")
                    nc.vector.tensor_copy(c2[:], ps_c2[:])

                    # applies: z <- z + X^(2^j) z   (lhsT = C_j)
                    for cj in (c0, c1, c2):
                        ps_ap = psp.tile([CH, DV], F32, name="ps_ap", tag="mm", bufs=3)
                        nc.tensor.matmul(ps_ap[:], cj[:], z[:])
                        z_new = opp.tile([CH, DV], MM_DT, name="z_new", tag="z", bufs=4)
                        nc.vector.tensor_tensor(z_new[:], ps_ap[:], z[:], _ALU.add)
                        z = z_new

                    # CQT = triu(kir @ qr^T, 0)
                    ps_cq = psp.tile([CH, CH], F32, name="ps_cq", tag="mm", bufs=3)
                    nc.tensor.matmul(ps_cq[:], kTi[:], qT[:])
                    cqt = opp.tile([CH, CH], MM_DT, name="cqt")
                    nc.vector.tensor_tensor(cqt[:], ps_cq[:], mask_ui[:], _ALU.mult)

                    # out = qr @ S + CQT^T @ z
                    ps_o = psp.tile([CH, DV], F32, name="ps_o", tag="ps_o", bufs=1)
                    nc.tensor.matmul(ps_o[:], qT[:], s_cur[:], start=True, stop=False)
                    nc.tensor.matmul(ps_o[:], cqt[:], z[:], start=False, stop=True)
                    o_sb = opp.tile([CH, DV], F32, name="o_sb")
                    nc.scalar.copy(o_sb[:], ps_o[:])
                    nc.sync.dma_start(dout[s, tsl, :], o_sb[:])

                    # state update: S' = E*(S + kir^T @ z)  [folded: Zs = E*z]
                    zs = opp.tile([CH, DV], MM_DT, name="zs")
                    nc.scalar.activation(
                        zs[:], z[:], _ACTF.Copy, scale=ET[c][:, s : s + 1]
                    )
                    ps_s = psp.tile([DK, DV], F32, name="ps_s", tag="ps_s", bufs=1)
                    nc.tensor.matmul(ps_s[:], kir[:], zs[:])
                    if c < N_CHUNKS - 1:
                        s_next = stp.tile([DK, DV], MM_DT, name="s_next")
                        nc.vector.scalar_tensor_tensor(
                            s_next[:], s_cur[:], ET[c][:, s : s + 1], ps_s[:],
                            op0=_ALU.mult, op1=_ALU.add,
                        )
                        s_cur = s_next
                    else:
                        s_fin = stp.tile([DK, DV], F32, name="s_fin")
                        nc.vector.scalar_tensor_tensor(
                            s_fin[:], s_cur[:], ET[c][:, s : s + 1], ps_s[:],
                            op0=_ALU.mult, op1=_ALU.add,
                        )
                        nc.sync.dma_start(dsn[s, :, :], s_fin[:])

    nc.compile()
    return nc


_NC_CACHE = {}


def _get_nc(n_slices):
    if n_slices not in _NC_CACHE:
        _NC_CACHE[n_slices] = build_nc(n_slices)
    return _NC_CACHE[n_slices]


def kernel(q, k, v, g, beta, last_recurrent_state):
    from concourse.bass_utils import run_bass_kernel_spmd

    qf = np.ascontiguousarray(q, np.float32).reshape(B * H, T, DK)
    kf = np.ascontiguousarray(k, np.float32).reshape(B * H, T, DK)
    vf = np.ascontiguousarray(v, np.float32).reshape(B * H, T, DV)
    gf = np.ascontiguousarray(g, np.float32).reshape(B * H, T)
    bf = np.ascontiguousarray(beta, np.float32).reshape(B * H, T)
    sf = np.ascontiguousarray(last_recurrent_state, np.float32).reshape(B * H, DK, DV)

    nc = _get_nc(N_SLICES)
    in_maps = []
    for i in range(N_CORES):
        sl = slice(i * N_SLICES, (i + 1) * N_SLICES)
        in_maps.append(
            {
                "q": qf[sl],
                "k": kf[sl],
                "v": vf[sl],
                "g": gf[sl],
                "beta": bf[sl],
                "s0": sf[sl],
            }
        )
    res = run_bass_kernel_spmd(nc, in_maps, list(range(N_CORES)))
    out = np.concatenate([res.results[i]["out"] for i in range(N_CORES)], axis=0)
    s_new = np.concatenate([res.results[i]["s_new"] for i in range(N_CORES)], axis=0)
    return np.concatenate([out.reshape(-1), s_new.reshape(-1)], axis=0)


# revision 7
# speedup vs baseline: 1.4197x; 1.4197x over previous
"""Trainium2 Bass kernel for nn_ChunkwiseRecurrentAttentionCell.

Math (per (b,h) slice; T=256, Dk=Dv=128):
    gc = cumsum(g);  A = tril(beta_i exp(gc_i-gc_j) k_i.k_j, -1)
    v_new = (I+A)^{-1} (beta v - beta exp(gc) (k @ S0))
    out   = exp(gc) (q@S0) + (tril(exp(gc_i-gc_j),0) * (q k^T)) @ v_new
    S_new = exp(gc_T) S0 + k^T (v_new * exp(gc_T - gc))

Implemented as a chunked recurrence (2 chunks of 128) so all per-chunk exp
ratios are bounded by e^6.4 (fp16-safe).  The triangular solve uses an
8-term Neumann product form  (I+X^4)(I+X^2)(I+X), X = -A_chunk, with dual
power chains (both X^p and its transpose built by matmuls from masked
scalings of the symmetric K K^T — no big transposes needed).  All matmul
operands are fp16 (PE runs fp16 at 1 cycle/row vs fp32's 4); accumulation
is fp32 in PSUM.  Relative error vs the fp32 reference ~ 4e-4.

Sharding: (B,H) flattened to 512 independent slices, 64 per core across
8 NeuronCores (data parallel, no collectives).
"""

import os
import numpy as np

import concourse.bass as bass
import concourse.mybir as mybir
from concourse import bacc
from concourse.tile import TileContext
from concourse.masks import (
    make_identity,
    make_lower_triangular,
    make_upper_triangular,
)

B, H, T, DK, DV = 16, 32, 256, 128, 128
N_CORES = 8
N_SLICES = (B * H) // N_CORES  # 64 per core
CH = 128  # chunk length
N_CHUNKS = T // CH
LEVELS = 3  # Neumann product-form levels -> 2^3 = 8 series terms

F32 = mybir.dt.float32
MM_DT = mybir.dt.float16

_ALU = mybir.AluOpType
_ACTF = mybir.ActivationFunctionType


def build_nc(n_slices: int = N_SLICES):
    nc = bacc.Bacc("TRN2", target_bir_lowering=False)

    dq = nc.dram_tensor("q", [n_slices, T, DK], F32, kind="ExternalInput")
    dk = nc.dram_tensor("k", [n_slices, T, DK], F32, kind="ExternalInput")
    dv = nc.dram_tensor("v", [n_slices, T, DV], F32, kind="ExternalInput")
    dg = nc.dram_tensor("g", [n_slices, T], F32, kind="ExternalInput")
    db = nc.dram_tensor("beta", [n_slices, T], F32, kind="ExternalInput")
    ds0 = nc.dram_tensor("s0", [n_slices, DK, DV], F32, kind="ExternalInput")
    dout = nc.dram_tensor("out", [n_slices, T, DV], F32, kind="ExternalOutput")
    dsn = nc.dram_tensor("s_new", [n_slices, DK, DV], F32, kind="ExternalOutput")

    with TileContext(nc) as tc:
        with (
            tc.tile_pool(name="const", bufs=1) as cpool,
            tc.tile_pool(name="io", bufs=3) as iop,
            tc.tile_pool(name="ops", bufs=3) as opp,
            tc.tile_pool(name="state", bufs=2) as stp,
            tc.tile_pool(name="ps", bufs=1, space="PSUM") as psp,
        ):
            # ---------------- constants ----------------
            ident16 = cpool.tile([128, 128], MM_DT)
            make_identity(nc, ident16)
            ident32 = cpool.tile([128, 128], F32)
            make_identity(nc, ident32)
            mask_sl = cpool.tile([128, 128], F32)  # strict lower ones
            make_lower_triangular(nc, mask_sl, val=1.0, diag=False)
            mask_su = cpool.tile([128, 128], F32)  # strict upper ones
            make_upper_triangular(nc, mask_su, val=1.0, diag=False)
            mask_ui = cpool.tile([128, 128], F32)  # upper ones incl diag
            make_upper_triangular(nc, mask_ui, val=1.0, diag=True)

            # ---------------- per-core setup: gate vectors ----------------
            gt = cpool.tile([n_slices, T], F32)
            nc.sync.dma_start(gt[:], dg[:])
            bt = cpool.tile([n_slices, T], F32)
            nc.sync.dma_start(bt[:], db[:])
            gct = cpool.tile([n_slices, T], F32)
            nc.vector.tensor_tensor_scan(
                gct[:], gt[:], gt[:], 0.0, op0=_ALU.add, op1=_ALU.bypass
            )
            gcl1 = cpool.tile([n_slices, CH], F32)
            nc.vector.tensor_scalar(
                gcl1[:], gct[:, CH : 2 * CH], gct[:, CH - 1 : CH], None,
                op0=_ALU.subtract,
            )

            # per chunk: r, 1/r, -beta*r  in [n_slices, CH]; then transpose to
            # [CH, n_slices] so columns are per-slice partition-scalars.
            rT, irT, nbrT, bT, ET = [], [], [], [], []
            for c in range(N_CHUNKS):
                gcl = gct[:, 0:CH] if c == 0 else gcl1[:]
                r_c = cpool.tile([n_slices, CH], F32, name=f"r_{c}")
                nc.scalar.activation(r_c[:], gcl, _ACTF.Exp)
                ir_c = cpool.tile([n_slices, CH], F32, name=f"ir_{c}")
                nc.scalar.activation(ir_c[:], gcl, _ACTF.Exp, scale=-1.0)
                nbr_c = cpool.tile([n_slices, CH], F32, name=f"nbr_{c}")
                nc.vector.scalar_tensor_tensor(
                    nbr_c[:],
                    bt[:, c * CH : (c + 1) * CH],
                    -1.0,
                    r_c[:],
                    op0=_ALU.mult,
                    op1=_ALU.mult,
                )
                outs = []
                for src, nm in (
                    (r_c[:], "rT"),
                    (ir_c[:], "irT"),
                    (nbr_c[:], "nbrT"),
                    (bt[:, c * CH : (c + 1) * CH], "bT"),
                ):
                    pst = psp.tile([CH, n_slices], F32, name=f"pst_{nm}{c}", tag="ps_t", bufs=3)
                    nc.tensor.transpose(pst[:], src, ident32[0:n_slices, 0:n_slices])
                    dst = cpool.tile([CH, n_slices], F32, name=f"{nm}_{c}")
                    nc.scalar.copy(dst[:], pst[:])
                    outs.append(dst)
                rT.append(outs[0])
                irT.append(outs[1])
                nbrT.append(outs[2])
                bT.append(outs[3])
                ps_e = psp.tile([1, n_slices], F32, name=f"ps_e{c}", tag="ps_t", bufs=3)
                nc.tensor.transpose(
                    ps_e[:], r_c[:, CH - 1 : CH], ident32[0:n_slices, 0:n_slices]
                )
                e_row = cpool.tile([1, n_slices], F32, name=f"e_row_{c}")
                nc.scalar.copy(e_row[:], ps_e[:])
                e_c = cpool.tile([CH, n_slices], F32, name=f"ET_{c}")
                nc.gpsimd.partition_broadcast(e_c[:], e_row[0:1, :])
                ET.append(e_c)

            # ---------------- main loop over slices ----------------
            for s in range(n_slices):
                s_cur = None
                for c in range(N_CHUNKS):
                    tsl = slice(c * CH, (c + 1) * CH)
                    q_c = iop.tile([CH, DK], F32, name="q_c")
                    nc.sync.dma_start(q_c[:], dq[s, tsl, :])
                    k_c = iop.tile([CH, DK], F32, name="k_c")
                    nc.sync.dma_start(k_c[:], dk[s, tsl, :])
                    v_c = iop.tile([CH, DV], F32, name="v_c")
                    nc.sync.dma_start(v_c[:], dv[s, tsl, :])
                    if c == 0:
                        s_f32 = iop.tile([DK, DV], F32, name="s_f32")
                        nc.sync.dma_start(s_f32[:], ds0[s, :, :])
                        s_cur = stp.tile([DK, DV], MM_DT, name="s_cur")
                        nc.gpsimd.tensor_copy(s_cur[:], s_f32[:])

                    # scaled copies (fp16)
                    qr = opp.tile([CH, DK], MM_DT, name="qr")
                    nc.scalar.activation(
                        qr[:], q_c[:], _ACTF.Copy, scale=rT[c][:, s : s + 1]
                    )
                    knbr = opp.tile([CH, DK], MM_DT, name="knbr")
                    nc.vector.tensor_scalar_mul(knbr[:], k_c[:], nbrT[c][:, s : s + 1])
                    kir = opp.tile([CH, DK], MM_DT, name="kir")
                    nc.vector.tensor_scalar_mul(kir[:], k_c[:], irT[c][:, s : s + 1])

                    # transposes (PE) + copies (ACT)
                    qT = opp.tile([DK, CH], MM_DT, name="qT")
                    kTn = opp.tile([DK, CH], MM_DT, name="kTn")
                    kTi = opp.tile([DK, CH], MM_DT, name="kTi")
                    for src, dst, nm in ((qr, qT, "q"), (knbr, kTn, "n"), (kir, kTi, "i")):
                        ps_t = psp.tile([DK, CH], MM_DT, name=f"ps_t{nm}", tag="ps_t", bufs=3)
                        nc.tensor.transpose(ps_t[:], src[:], ident16[:])
                        nc.scalar.copy(dst[:], ps_t[:])

                    # Y = beta*v + (knbr @ S)     [= beta*v - beta*r*(k@S)]
                    ps_y = psp.tile([CH, DV], F32, name="ps_y", tag="mm", bufs=3)
                    nc.tensor.matmul(ps_y[:], kTn[:], s_cur[:])
                    z = opp.tile([CH, DV], MM_DT, name="z_it", tag="z", bufs=4)
                    nc.vector.scalar_tensor_tensor(
                        z[:], v_c[:], bT[c][:, s : s + 1], ps_y[:],
                        op0=_ALU.mult, op1=_ALU.add,
                    )

                    # B0 = -A = strict_tril(knbr @ kir^T); C0 = B0^T
                    ps_a = psp.tile([CH, CH], F32, name="ps_a", tag="mm", bufs=3)
                    nc.tensor.matmul(ps_a[:], kTn[:], kTi[:])
                    b0 = opp.tile([CH, CH], MM_DT, name="b0")
                    nc.vector.tensor_tensor(b0[:], ps_a[:], mask_sl[:], _ALU.mult)
                    ps_at = psp.tile([CH, CH], F32, name="ps_at", tag="mm", bufs=3)
                    nc.tensor.matmul(ps_at[:], kTi[:], kTn[:])
                    c0 = opp.tile([CH, CH], MM_DT, name="c0")
                    nc.vector.tensor_tensor(c0[:], ps_at[:], mask_su[:], _ALU.mult)

                    # dual chain: B1 = B0@B0, C1 = C0@C0, C2 = C1@C1
                    ps_b1 = psp.tile([CH, CH], F32, name="ps_b1", tag="mm", bufs=3)
                    nc.tensor.matmul(ps_b1[:], c0[:], b0[:])
                    b1 = opp.tile([CH, CH], MM_DT, name="b1")
                    nc.scalar.copy(b1[:], ps_b1[:])
                    ps_c1 = psp.tile([CH, CH], F32, name="ps_c1", tag="mm", bufs=3)
                    nc.tensor.matmul(ps_c1[:], b0[:], c0[:])
                    c1 = opp.tile([CH, CH], MM_DT, name="c1")
                    nc.scalar.copy(c1[:], ps_c1[:])
                    ps_c2 = psp.tile([CH, CH], F32, name="ps_c2", tag="mm", bufs=3)
                    nc.tensor.matmul(ps_c2[:], b1[:], c1[:])
                    c2 = opp.tile([CH, CH], MM_DT, name="c2")
                    nc.vector.tensor_copy(c2[:], ps_c2[:])

                    # applies: z <- z + X^(2^j) z   (lhsT = C_j)
                    for cj in (c0, c1, c2):
                        ps_ap = psp.tile([CH, DV], F32, name="ps_ap", tag="mm", bufs=3)
                        nc.tensor.matmul(ps_ap[:], cj[:], z[:])
                        z_new = opp.tile([CH, DV], MM_DT, name="z_new", tag="z", bufs=4)
                        nc.vector.tensor_tensor(z_new[:], ps_ap[:], z[:], _ALU.add)
                        z = z_new

                    # CQT = triu(kir @ qr^T, 0)
                    ps_cq = psp.tile([CH, CH], F32, name="ps_cq", tag="mm", bufs=3)
                    nc.tensor.matmul(ps_cq[:], kTi[:], qT[:])
                    cqt = opp.tile([CH, CH], MM_DT, name="cqt")
                    nc.vector.tensor_tensor(cqt[:], ps_cq[:], mask_ui[:], _ALU.mult)

                    # out = qr @ S + CQT^T @ z
                    ps_o = psp.tile([CH, DV], F32, name="ps_o", tag="ps_o", bufs=1)
                    nc.tensor.matmul(ps_o[:], qT[:], s_cur[:], start=True, stop=False)
                    nc.tensor.matmul(ps_o[:], cqt[:], z[:], start=False, stop=True)
                    o_sb = opp.tile([CH, DV], F32, name="o_sb")
                    nc.scalar.copy(o_sb[:], ps_o[:])
                    nc.sync.dma_start(dout[s, tsl, :], o_sb[:])

                    # state update: S' = E*(S + kir^T @ z)  [folded: Zs = E*z]
                    zs = opp.tile([CH, DV], MM_DT, name="zs")
                    nc.scalar.activation(
                        zs[:], z[:], _ACTF.Copy, scale=ET[c][:, s : s + 1]
                    )
                    ps_s = psp.tile([DK, DV], F32, name="ps_s", tag="ps_s", bufs=1)
                    nc.tensor.matmul(ps_s[:], kir[:], zs[:])
                    if c < N_CHUNKS - 1:
                        s_next = stp.tile([DK, DV], MM_DT, name="s_next")
                        nc.vector.scalar_tensor_tensor(
                            s_next[:], s_cur[:], ET[c][:, s : s + 1], ps_s[:],
                            op0=_ALU.mult, op1=_ALU.add,
                        )
                        s_cur = s_next
                    else:
                        s_fin = stp.tile([DK, DV], F32, name="s_fin")
                        nc.vector.scalar_tensor_tensor(
                            s_fin[:], s_cur[:], ET[c][:, s : s + 1], ps_s[:],
                            op0=_ALU.mult, op1=_ALU.add,
                        )
                        nc.sync.dma_start(dsn[s, :, :], s_fin[:])

    nc.compile()
    return nc


_NC_CACHE = {}


def _get_nc(n_slices):
    if n_slices not in _NC_CACHE:
        _NC_CACHE[n_slices] = build_nc(n_slices)
    return _NC_CACHE[n_slices]


def kernel(q, k, v, g, beta, last_recurrent_state):
    from concourse.bass_utils import run_bass_kernel_spmd

    qf = np.ascontiguousarray(q, np.float32).reshape(B * H, T, DK)
    kf = np.ascontiguousarray(k, np.float32).reshape(B * H, T, DK)
    vf = np.ascontiguousarray(v, np.float32).reshape(B * H, T, DV)
    gf = np.ascontiguousarray(g, np.float32).reshape(B * H, T)
    bf = np.ascontiguousarray(beta, np.float32).reshape(B * H, T)
    sf = np.ascontiguousarray(last_recurrent_state, np.float32).reshape(B * H, DK, DV)

    nc = _get_nc(N_SLICES)
    in_maps = []
    for i in range(N_CORES):
        sl = slice(i * N_SLICES, (i + 1) * N_SLICES)
        in_maps.append(
            {
                "q": qf[sl],
                "k": kf[sl],
                "v": vf[sl],
                "g": gf[sl],
                "beta": bf[sl],
                "s0": sf[sl],
            }
        )
    res = run_bass_kernel_spmd(nc, in_maps, list(range(N_CORES)))
    out = np.concatenate([res.results[i]["out"] for i in range(N_CORES)], axis=0)
    s_new = np.concatenate([res.results[i]["s_new"] for i in range(N_CORES)], axis=0)
    return np.concatenate([out.reshape(-1), s_new.reshape(-1)], axis=0)
